# revision 1
# baseline (speedup 1.0000x reference)
"""Trainium2 Bass kernel for nn_DMHA_3255585210402 (retrieval_knn DMHA).

Key algebraic fact: TOPK == NVK == 4, so jax.lax.top_k over the size-4 v_keys
axis selects *all* entries; the gather+sum over (DVH, TOPK) therefore reduces
to a constant vector c = 2 * v_embed[0:4].sum(0), and the whole
compute_value_states branch collapses to  v = x * c  (verified: 1.4e-7 rel).

So the module is a causal MHA layer (B=2, H=16, T=2048, HD=128, D=2048) with
elementwise-scaled V.  Sharding: 8 cores = 2 batches x 4 head-groups.  Each
core computes, for its batch b and 4 heads:
  qT/kT projections (transposed layout, feature-on-partition),
  causal softmax attention in transposed score layout (sT[tk, tq]),
  the partial output projection  outT_g = Wo[:, gsl]-slice.T @ oT.
Host sums the 4 head-group partials per batch and adds bo.

The c scale rides the per-partition scalar of the normalize multiply
(o = c[p] * (x_g.T @ w) * recip[tq]), so V is never materialized.
All matmuls run as float32r; softmax denominators use the ones-column
matmul for the partition reduction and reciprocal_approx_fast + a
DMA row-broadcast so the PE never waits on the normalization chain.
"""

import math

import numpy as np

import concourse.bass as bass
import concourse.mybir as mybir
import concourse.tile as tile
from concourse import bacc
from concourse.bass_utils import run_bass_kernel_spmd

B, T, D = 2, 2048, 2048
H, HD = 16, 128
G = 4              # head-groups (cores per batch)
GH = H // G        # heads per core
GF = GH * HD       # projected features per core (512)
NCORES = 8
P = 128            # partitions
TQ = 512           # tq chunk width (psum bank / fp32 moving max)
F32 = mybir.dt.float32
F32R = mybir.dt.float32r

DK = D // P        # 16 contraction chunks for projections
NTQ = T // TQ      # 4 query chunks
NTK = T // P       # 16 key chunks


def _body(tc, xT, xg, wqT, wkT, woT, cT, bqT, bkT, ones, out):
    nc = tc.nc
    rsqrt_hd = 1.0 / math.sqrt(HD)
    mult = mybir.AluOpType.mult

    with (
        tc.tile_pool(name="const", bufs=1) as const,
        tc.tile_pool(name="res1", bufs=1) as res1,
    ):
        # preload the gpsimd library that partition_broadcast needs so the
        # ~11us library DMA happens during phase A, not at first use
        from concourse import library_config
        with tc.high_priority():
            nc.gpsimd.load_library(library_config.attn)
        qT_sb = res1.tile([P, GH, T], F32R)   # q, transposed per head
        kT_sb = res1.tile([P, GH, T], F32R)

        # --- phase A: q/k projections, transposed layout ---
        with (
            tc.tile_pool(name="wqk", bufs=1) as wqk,
            tc.tile_pool(name="xt", bufs=20) as xtp,
            tc.tile_pool(name="psA", bufs=8, space="PSUM") as psA,
        ):
            wq_sb = wqk.tile([P, DK, GF], F32R)
            wk_sb = wqk.tile([P, DK, GF], F32R)
            wqr = wqT.rearrange("(n p) f -> p n f", p=P)
            wkr = wkT.rearrange("(n p) f -> p n f", p=P)
            xts0 = []
            for dk in range(DK):
                nc.sync.dma_start(out=wq_sb[:, dk, :], in_=wqr[:, dk, :])
                nc.sync.dma_start(out=wk_sb[:, dk, :], in_=wkr[:, dk, :])
                xt0 = xtp.tile([P, TQ], F32R, name="xt")
                nc.sync.dma_start(
                    out=xt0, in_=xT[dk * P : (dk + 1) * P, 0:TQ]
                )
                xts0.append(xt0)

            # small constants (needed from the first psum copy onward)
            ones_sb = const.tile([P, P], F32R)
            nc.sync.dma_start(out=ones_sb, in_=ones)
            bq_sb = const.tile([HD, GH], F32)
            nc.sync.dma_start(out=bq_sb, in_=bqT)
            bk_sb = const.tile([HD, GH], F32)
            nc.sync.dma_start(out=bk_sb, in_=bkT)
            cT_sb = const.tile([HD, GH], F32)
            nc.sync.dma_start(out=cT_sb, in_=cT)

            for tci in range(NTQ):
                tsl = slice(tci * TQ, (tci + 1) * TQ)
                ps = [
                    psA.tile([P, TQ], F32, name="psA_t", tag="psA_t")
                    for _ in range(2 * GH)
                ]
                for dk in range(DK):
                    if tci == 0:
                        xt = xts0[dk]
                    else:
                        xt = xtp.tile([P, TQ], F32R, name="xt")
                        nc.sync.dma_start(
                            out=xt, in_=xT[dk * P : (dk + 1) * P, tsl]
                        )
                    for w, w_sb in enumerate((wq_sb, wk_sb)):
                        for h in range(GH):
                            nc.tensor.matmul(
                                ps[w * GH + h],
                                w_sb[:, dk, h * HD : (h + 1) * HD],
                                xt,
                                start=(dk == 0),
                                stop=(dk == DK - 1),
                            )
                for w, dstT, bias in ((0, qT_sb, bq_sb), (1, kT_sb, bk_sb)):
                    for h in range(GH):
                        nc.scalar.activation(
                            dstT[:, h, tsl],
                            ps[w * GH + h],
                            mybir.ActivationFunctionType.Identity,
                            bias=bias[:, h : h + 1],
                        )

        # --- phases B+C interleaved over query chunks ---
        with (
            tc.tile_pool(name="res2", bufs=1) as res2,
            tc.tile_pool(name="wt", bufs=6) as wtp,
            tc.tile_pool(name="pr", bufs=3) as prp,
            tc.tile_pool(name="small", bufs=4) as smp,
            tc.tile_pool(name="ct", bufs=4) as ctp,
            tc.tile_pool(name="psS", bufs=4, space="PSUM") as psS,
            tc.tile_pool(name="psO", bufs=2, space="PSUM") as psO,
            tc.tile_pool(name="psSum", bufs=2, space="PSUM") as psSum,
        ):
            xg_sb = res2.tile([P, NTK, GF], F32R)  # x[:, gsl] chunked by tk
            for i in range(NTK):
                nc.sync.dma_start(
                    out=xg_sb[:, i, :], in_=xg[i * P : (i + 1) * P, :]
                )
            oT_sb = res2.tile([P, GH, T], F32R)   # attention out, transposed
            wo_sb = res2.tile([P, GH, D], F32R)   # Wo[:, gsl].T chunked
            wor = woT.rearrange("(m p) d -> p m d", p=P)
            for m in range(GH):
                nc.sync.dma_start(out=wo_sb[:, m, :], in_=wor[:, m, :])

            pending = None
            for j in range(NTQ):
                qsl = slice(j * TQ, (j + 1) * TQ)
                nkk = (j + 1) * (TQ // P)  # causal: tk chunks needed
                # B: attention for each head on this query chunk
                for h in range(GH):
                    ps_o = psO.tile([P, TQ], F32, name="ps_o")
                    ps_sum = psSum.tile([1, TQ], F32, name="ps_sum")
                    wt_prev = None
                    for i in range(nkk):
                        ps_s = psS.tile([P, TQ], F32, name="ps_s", tag="ps_s")
                        nc.tensor.matmul(
                            ps_s,
                            kT_sb[:, h, i * P : (i + 1) * P],
                            qT_sb[:, h, qsl],
                            start=True,
                            stop=True,
                        )
                        wt = wtp.tile([P, TQ], F32R, name="wt")
                        nc.scalar.activation(
                            wt, ps_s, mybir.ActivationFunctionType.Exp,
                            scale=rsqrt_hd,
                        )
                        g = i - (TQ // P) * j
                        if g >= 0:  # diagonal tile: zero where tk > tq
                            nc.gpsimd.affine_select(
                                out=wt,
                                in_=wt,
                                pattern=[[1, TQ]],
                                compare_op=mybir.AluOpType.is_ge,
                                fill=0.0,
                                base=-(P * g),
                                channel_multiplier=-1,
                            )
                        nc.tensor.matmul(
                            ps_o,
                            xg_sb[:, i, h * HD : (h + 1) * HD],
                            wt,
                            start=(i == 0), stop=(i == nkk - 1),
                        )
                        # colsum: DVE pair-sums halve the PE's ones-matmuls
                        if i % 2 == 1:
                            wpair = prp.tile([P, TQ], F32R, name="wpair")
                            nc.vector.tensor_add(wpair, wt_prev, wt)
                            nc.tensor.matmul(
                                ps_sum, ones_sb[:, 0:1], wpair,
                                start=(i == 1), stop=(i == nkk - 1),
                            )
                        wt_prev = wt
                    # normalization (1/colsum -> partition broadcast ->
                    # (ps_o*c)*recip) is deferred one head so neither the
                    # gpsimd queue nor the PE ever waits on the chain
                    if pending is not None:
                        _emit_normalize(nc, smp, wtp, oT_sb, cT_sb, mult,
                                        *pending)
                    pending = (h, j, ps_o, ps_sum)
                # C: output projection, deferred one chunk so the PE
                # has B(j) queued while C(j-1)'s oT dependencies settle
                if j > 0:
                    _emit_outproj(nc, psS, ctp, wo_sb, oT_sb, out, j - 1)
            _emit_normalize(nc, smp, wtp, oT_sb, cT_sb, mult, *pending)
            _emit_outproj(nc, psS, ctp, wo_sb, oT_sb, out, NTQ - 1)


def _emit_normalize(nc, smp, wtp, oT_sb, cT_sb, mult, h, j, ps_o, ps_sum):
    """1/colsum on one partition, gpsimd partition broadcast, then
    (ps_o * c[p]) * recip in one DVE pass."""
    qsl = slice(j * TQ, (j + 1) * TQ)
    recip = smp.tile([1, TQ], F32, name="recip")
    nc.vector.reciprocal_approx_fast(out=recip, in_=ps_sum)
    rb = wtp.tile([P, TQ], F32, name="rb")
    nc.gpsimd.partition_broadcast(rb, recip)
    nc.vector.scalar_tensor_tensor(
        out=oT_sb[:, h, qsl],
        in0=ps_o,
        scalar=cT_sb[:, h : h + 1],
        in1=rb,
        op0=mult,
        op1=mult,
    )


def _emit_outproj(nc, psS, ctp, wo_sb, oT_sb, out, j):
    qsl = slice(j * TQ, (j + 1) * TQ)
    for dk in range(DK):
        ps = psS.tile([P, TQ], F32, name="psC_t", tag="ps_s")
        for m in range(GH):
            nc.tensor.matmul(
                ps,
                wo_sb[:, m, dk * P : (dk + 1) * P],
                oT_sb[:, m, qsl],
                start=(m == 0),
                stop=(m == GH - 1),
            )
        ct = ctp.tile([P, TQ], F32, name="ct")
        nc.scalar.copy(ct, ps)
        nc.sync.dma_start(out=out[dk * P : (dk + 1) * P, qsl], in_=ct)


def build_program():
    nc = bacc.Bacc(
        "TRN2", target_bir_lowering=False, debug=False, num_devices=NCORES
    )
    f = F32
    xT = nc.dram_tensor("xT", [D, T], F32R, kind="ExternalInput").ap()
    xg = nc.dram_tensor("xg", [T, GF], F32R, kind="ExternalInput").ap()
    wqT = nc.dram_tensor("wqT", [D, GF], F32R, kind="ExternalInput").ap()
    wkT = nc.dram_tensor("wkT", [D, GF], F32R, kind="ExternalInput").ap()
    woT = nc.dram_tensor("woT", [GF, D], F32R, kind="ExternalInput").ap()
    cT = nc.dram_tensor("cT", [HD, GH], f, kind="ExternalInput").ap()
    bqT = nc.dram_tensor("bqT", [HD, GH], f, kind="ExternalInput").ap()
    bkT = nc.dram_tensor("bkT", [HD, GH], f, kind="ExternalInput").ap()
    ones = nc.dram_tensor("ones", [P, P], F32R, kind="ExternalInput").ap()
    out = nc.dram_tensor("out", [D, T], f, kind="ExternalOutput").ap()

    with tile.TileContext(nc) as tc:
        _body(tc, xT, xg, wqT, wkT, woT, cT, bqT, bkT, ones, out)
    nc.compile()
    return nc


def _causal_masks() -> np.ndarray:
    """mask[g][p, f] = 1 iff tk <= tq for boundary tile offset g*128."""
    p = np.arange(P)[:, None]
    f = np.arange(TQ)[None, :]
    return np.stack(
        [(f >= p + g * P).astype(np.float32) for g in range(G)], axis=0
    )


_NC_CACHE = None
LAST_RESULT = None
TRACE = False


def kernel(x, Wq, bq, Wk, bk, Wvq, bvq, v_keys, v_embed, Wo, bo):
    global _NC_CACHE, LAST_RESULT
    x = np.asarray(x, np.float32)
    Wq = np.asarray(Wq, np.float32)
    bq = np.asarray(bq, np.float32)
    Wk = np.asarray(Wk, np.float32)
    bk = np.asarray(bk, np.float32)
    v_embed = np.asarray(v_embed, np.float32)
    Wo = np.asarray(Wo, np.float32)
    bo = np.asarray(bo, np.float32)

    c = 2.0 * v_embed[:G].sum(axis=0)
    in_maps = []
    for core in range(NCORES):
        b, g = divmod(core, G)
        gsl = slice(g * GF, (g + 1) * GF)
        in_maps.append(
            {
                "xT": np.ascontiguousarray(x[b].T),
                "xg": np.ascontiguousarray(x[b][:, gsl]),
                "wqT": np.ascontiguousarray(Wq[gsl, :].T),
                "wkT": np.ascontiguousarray(Wk[gsl, :].T),
                "woT": np.ascontiguousarray(Wo[:, gsl].T),
                "cT": np.ascontiguousarray(c[gsl].reshape(GH, HD).T),
                "bqT": np.ascontiguousarray(bq[gsl].reshape(GH, HD).T),
                "bkT": np.ascontiguousarray(bk[gsl].reshape(GH, HD).T),
                "ones": np.ones((P, P), np.float32),
            }
        )

    if _NC_CACHE is None:
        _NC_CACHE = build_program()
    res = run_bass_kernel_spmd(
        _NC_CACHE, in_maps, list(range(NCORES)), trace=TRACE
    )
    LAST_RESULT = res

    out = np.zeros((B, T, D), np.float32)
    for core in range(NCORES):
        b = core // G
        out[b] += res.results[core]["out"].T
    out += bo[None, None, :]
    return out


if __name__ == "__main__":
    nc = build_program()
    print("built ok")



# revision 6
# speedup vs baseline: 1.1248x; 1.1248x over previous
"""Trainium2 Bass kernel for nn_DMHA_3255585210402 (retrieval_knn DMHA).

Key algebraic fact: TOPK == NVK == 4, so jax.lax.top_k over the size-4 v_keys
axis selects *all* entries; the gather+sum over (DVH, TOPK) reduces to a
constant vector c = 2 * v_embed[0:4].sum(0), and compute_value_states
collapses to  v = x * c.

So the module is causal MHA (B=2, H=16, T=2048, HD=128, D=2048) with
elementwise-scaled V.  Sharding: 8 cores = 2 batches x 4 head-groups.

v2 design (vs v1):
  * all matmul operands bf16 (psum accumulation stays f32) - halves DMA
    and SBUF, and bf16 runs 1 cycle/row at ANY moving width (fp32r needs
    >=256), enabling fine-grained causal tiles.
  * diagonal 512-blocks computed at widths 512/384/256/128 instead of 4x512.
  * triangular mask applied by DVE tensor_mul with a [128,128] mask tile
    (gpsimd affine_select was on the exp->o-matmul critical path).
  * softmax denominators: off-diagonal chunks pair/quad-summed on DVE then
    one ones-matmul per quad; diagonal chunks get per-chunk ones-matmuls.
  * outproj psum->sbuf copies moved from Scalar to DVE so exp never queues
    behind them.
  * scores matmuls emitted with a 3-chunk skew ahead of o-matmuls to hide
    exp latency in the in-order PE queue.
  * DMA batching: Wq||Wk fused per-dk chunks (early), single-issue batched
    transfers for x (tci>=1), xg, Wo; output staged in SBUF and written as
    2 DMAs per query chunk (sync-engine DMA issue costs ~650ns each).
"""

import math

import numpy as np
import ml_dtypes

import concourse.bass as bass
import concourse.mybir as mybir
import concourse.tile as tile
from concourse import bacc
from concourse.bass_utils import run_bass_kernel_spmd

B, T, D = 2, 2048, 2048
H, HD = 16, 128
G = 4              # head-groups (cores per batch)
GH = H // G        # heads per core
GF = GH * HD       # projected features per core (512)
NCORES = 8
P = 128            # partitions
TQ = 512           # tq chunk width (psum bank / fp32 moving max)
F32 = mybir.dt.float32
BF16 = mybir.dt.bfloat16

DK = D // P        # 16 contraction chunks for projections
NTQ = T // TQ      # 4 query chunks
NTK = T // P       # 16 key chunks
SKEW = 3           # scores-ahead-of-o software pipeline depth

BF = ml_dtypes.bfloat16


def _body(tc, xT, xg, wqk, woT, cT, bqkT, ones, tri, out):
    nc = tc.nc
    rsqrt_hd = 1.0 / math.sqrt(HD)
    mult = mybir.AluOpType.mult

    with (
        tc.tile_pool(name="const", bufs=1) as const,
        tc.tile_pool(name="res1", bufs=1) as res1,
    ):
        # preload the gpsimd library that partition_broadcast needs so the
        # ~11us library DMA happens during phase A, not at first use
        from concourse import library_config
        with tc.high_priority():
            nc.gpsimd.load_library(library_config.attn)
        qT_sb = res1.tile([P, GH, T], BF16)   # q, transposed per head
        kT_sb = res1.tile([P, GH, T], BF16)
        # phase-B residents, DMA'd during phase A
        xg_sb = res1.tile([P, NTK, GF], BF16)   # x[:, gsl] chunked by tk
        wo_sb = res1.tile([P, GH, D], BF16)     # Wo[:, gsl].T chunked

        # --- phase A: q/k projections, transposed layout ---
        with (
            tc.tile_pool(name="wqk", bufs=1) as wqkp,
            tc.tile_pool(name="xt", bufs=2) as xtp,
            tc.tile_pool(name="psA", bufs=8, space="PSUM") as psA,
        ):
            wqk_sb = wqkp.tile([P, DK, 2, GF], BF16)
            xts = [xtp.tile([P, DK, TQ], BF16, name="xt") for _ in range(2)]
            # tci=0: per-dk chunked DMAs so the first matmul starts early
            for dk in range(DK):
                nc.sync.dma_start(out=wqk_sb[:, dk], in_=wqk[:, dk])
                nc.sync.dma_start(
                    out=xts[0][:, dk, :], in_=xT[:, dk, 0:TQ]
                )

            # small constants (single batched-issue DMAs)
            ones_sb = const.tile([P, 1], BF16)
            nc.sync.dma_start(out=ones_sb, in_=ones)
            tri_sb = const.tile([P, P], BF16)
            nc.sync.dma_start(out=tri_sb, in_=tri)
            bqk_sb = const.tile([HD, 2, GH], F32)
            nc.sync.dma_start(out=bqk_sb, in_=bqkT)
            cT_sb = const.tile([HD, GH], F32)
            nc.sync.dma_start(out=cT_sb, in_=cT)

            # batched prefetches for later phases (one issue each)
            nc.sync.dma_start(out=xts[1], in_=xT[:, :, TQ : 2 * TQ])
            nc.sync.dma_start(out=xg_sb, in_=xg)
            nc.sync.dma_start(out=wo_sb, in_=woT)

            for tci in range(NTQ):
                tsl = slice(tci * TQ, (tci + 1) * TQ)
                xt = xts[tci % 2]
                ps = [
                    psA.tile([P, TQ], F32, name="psA_t", tag="psA_t")
                    for _ in range(2 * GH)
                ]
                for dk in range(DK):
                    for w in range(2):
                        for h in range(GH):
                            nc.tensor.matmul(
                                ps[w * GH + h],
                                wqk_sb[:, dk, w, h * HD : (h + 1) * HD],
                                xt[:, dk, :],
                                start=(dk == 0),
                                stop=(dk == DK - 1),
                            )
                # prefetch next x chunk (single batched issue)
                if tci + 2 < NTQ:
                    nsl = slice((tci + 2) * TQ, (tci + 3) * TQ)
                    nc.sync.dma_start(out=xt, in_=xT[:, :, nsl])
                for w, dstT in ((0, qT_sb), (1, kT_sb)):
                    for h in range(GH):
                        nc.scalar.activation(
                            dstT[:, h, tsl],
                            ps[w * GH + h],
                            mybir.ActivationFunctionType.Identity,
                            bias=bqk_sb[:, w, h : h + 1],
                        )

        # --- phases B+C interleaved over query chunks ---
        with (
            tc.tile_pool(name="res2", bufs=1) as res2,
            tc.tile_pool(name="wt", bufs=8) as wtp,
            tc.tile_pool(name="rb", bufs=2) as rbp,
            tc.tile_pool(name="pr", bufs=6) as prp,
            tc.tile_pool(name="small", bufs=4) as smp,
            tc.tile_pool(name="stg", bufs=2) as stgp,
            tc.tile_pool(name="psS", bufs=4, space="PSUM") as psS,
            tc.tile_pool(name="psO", bufs=2, space="PSUM") as psO,
            tc.tile_pool(name="psSum", bufs=2, space="PSUM") as psSum,
        ):
            oT_sb = res2.tile([P, GH, T], BF16)   # attention out, transposed

            pending = None
            stage_prev = None
            for j in range(NTQ):
                qsl = slice(j * TQ, (j + 1) * TQ)
                nkk = (j + 1) * (TQ // P)  # causal: tk chunks needed
                ndiag = TQ // P            # last 4 chunks are diagonal
                noff = nkk - ndiag
                for h in range(GH):
                    ps_o = psO.tile([P, TQ], F32, name="ps_o")
                    ps_sum = psSum.tile([1, TQ], F32, name="ps_sum")
                    wts = [None] * nkk
                    subs = [None] * nkk
                    emitted_o = 0

                    def emit_scores(i):
                        """scores matmul + exp (+ mask on diagonal)."""
                        g = i - noff
                        if g >= 0:
                            sub = slice(g * P, TQ)
                            w = TQ - g * P
                        else:
                            sub = slice(0, TQ)
                            w = TQ
                        ps_s = psS.tile(
                            [P, TQ], F32, name="ps_s", tag="ps_s"
                        )
                        nc.tensor.matmul(
                            ps_s[:, sub],
                            kT_sb[:, h, i * P : (i + 1) * P],
                            qT_sb[:, h, j * TQ + (TQ - w) : (j + 1) * TQ],
                            start=True,
                            stop=True,
                        )
                        wt = wtp.tile([P, TQ], BF16, name="wt")
                        nc.scalar.activation(
                            wt[:, sub], ps_s[:, sub],
                            mybir.ActivationFunctionType.Exp,
                            scale=rsqrt_hd,
                        )
                        if g >= 0:  # triangular mask on leading 128 cols
                            lead = slice(g * P, (g + 1) * P)
                            nc.vector.tensor_mul(
                                wt[:, lead], wt[:, lead], tri_sb
                            )
                        wts[i] = wt
                        subs[i] = sub

                    def emit_o(i):
                        """accumulate o-matmul + colsum contributions."""
                        sub = subs[i]
                        nc.tensor.matmul(
                            ps_o[:, sub],
                            xg_sb[:, i, h * HD : (h + 1) * HD],
                            wts[i][:, sub],
                            start=(i == 0),
                            stop=(i == nkk - 1),
                        )
                        g = i - noff
                        if g < 0:
                            # off-diagonal: quad-group for the colsum
                            if i % 4 == 3:
                                t0 = prp.tile([P, TQ], BF16, name="pr")
                                t1 = prp.tile([P, TQ], BF16, name="pr")
                                q0 = prp.tile([P, TQ], BF16, name="pr")
                                nc.vector.tensor_add(
                                    t0, wts[i - 3], wts[i - 2]
                                )
                                nc.vector.tensor_add(
                                    t1, wts[i - 1], wts[i]
                                )
                                nc.vector.tensor_add(q0, t0, t1)
                                nc.tensor.matmul(
                                    ps_sum, ones_sb, q0,
                                    start=(i == 3), stop=False,
                                )
                        else:
                            # diagonal: per-chunk ones-matmul at its width
                            nc.tensor.matmul(
                                ps_sum[:, sub], ones_sb, wts[i][:, sub],
                                start=(j == 0 and g == 0),
                                stop=(g == ndiag - 1),
                            )

                    for i in range(nkk):
                        emit_scores(i)
                        if i >= SKEW:
                            emit_o(emitted_o)
                            emitted_o += 1
                    while emitted_o < nkk:
                        emit_o(emitted_o)
                        emitted_o += 1

                    # normalization deferred one head so neither gpsimd nor
                    # the PE ever waits on the recip chain
                    if pending is not None:
                        _emit_normalize(nc, smp, rbp, oT_sb, cT_sb, mult,
                                        *pending)
                    pending = (h, j, ps_o, ps_sum)
                # C: output projection, deferred one chunk so the PE
                # has B(j) queued while C(j-1)'s oT dependencies settle
                if j > 0:
                    _emit_outproj(nc, psS, stgp, wo_sb, oT_sb, out, j - 1)
            _emit_normalize(nc, smp, rbp, oT_sb, cT_sb, mult, *pending)
            _emit_outproj(nc, psS, stgp, wo_sb, oT_sb, out, NTQ - 1)


def _emit_normalize(nc, smp, rbp, oT_sb, cT_sb, mult, h, j, ps_o, ps_sum):
    """1/colsum on one partition, gpsimd partition broadcast, then
    (ps_o * c[p]) * recip in one DVE pass."""
    qsl = slice(j * TQ, (j + 1) * TQ)
    recip = smp.tile([1, TQ], F32, name="recip")
    nc.vector.reciprocal_approx_fast(out=recip, in_=ps_sum)
    rb = rbp.tile([P, TQ], F32, name="rb")
    nc.gpsimd.partition_broadcast(rb, recip)
    nc.vector.scalar_tensor_tensor(
        out=oT_sb[:, h, qsl],
        in0=ps_o,
        scalar=cT_sb[:, h : h + 1],
        in1=rb,
        op0=mult,
        op1=mult,
    )


def _emit_outproj(nc, psS, stgp, wo_sb, oT_sb, out, j):
    qsl = slice(j * TQ, (j + 1) * TQ)
    stage = stgp.tile([P, DK, TQ], BF16, name="stage")
    for dk in range(DK):
        ps = psS.tile([P, TQ], F32, name="psC_t", tag="ps_s")
        for m in range(GH):
            nc.tensor.matmul(
                ps,
                wo_sb[:, m, dk * P : (dk + 1) * P],
                oT_sb[:, m, qsl],
                start=(m == 0),
                stop=(m == GH - 1),
            )
        nc.vector.tensor_copy(stage[:, dk, :], ps)
        if dk == DK // 2 - 1:
            nc.sync.dma_start(
                out=out[:, 0 : DK // 2, qsl], in_=stage[:, 0 : DK // 2, :]
            )
    nc.sync.dma_start(
        out=out[:, DK // 2 : DK, qsl], in_=stage[:, DK // 2 : DK, :]
    )
    return stage


def build_program():
    nc = bacc.Bacc(
        "TRN2", target_bir_lowering=False, debug=False, num_devices=NCORES
    )
    f = F32
    # xT: [128, DK, T] bf16 (feature-chunked, feature-on-partition)
    xT = nc.dram_tensor("xT", [P, DK, T], BF16, kind="ExternalInput").ap()
    # xg: [128, NTK, GF] bf16 (time-chunked slice of x for V)
    xg = nc.dram_tensor("xg", [P, NTK, GF], BF16, kind="ExternalInput").ap()
    # wqk: [128, DK, 2, GF] bf16 (fused Wq/Wk, transposed, chunked)
    wqk = nc.dram_tensor(
        "wqk", [P, DK, 2, GF], BF16, kind="ExternalInput"
    ).ap()
    # woT: [128, GH, D] bf16
    woT = nc.dram_tensor("woT", [P, GH, D], BF16, kind="ExternalInput").ap()
    cT = nc.dram_tensor("cT", [HD, GH], f, kind="ExternalInput").ap()
    bqkT = nc.dram_tensor("bqkT", [HD, 2, GH], f, kind="ExternalInput").ap()
    ones = nc.dram_tensor("ones", [P, 1], BF16, kind="ExternalInput").ap()
    tri = nc.dram_tensor("tri", [P, P], BF16, kind="ExternalInput").ap()
    # out: [128, DK, T] bf16 (row-chunked [D, T])
    out = nc.dram_tensor("out", [P, DK, T], BF16, kind="ExternalOutput").ap()

    with tile.TileContext(nc) as tc:
        _body(tc, xT, xg, wqk, woT, cT, bqkT, ones, tri, out)
    nc.compile()
    return nc


_NC_CACHE = None
LAST_RESULT = None
TRACE = False


def kernel(x, Wq, bq, Wk, bk, Wvq, bvq, v_keys, v_embed, Wo, bo):
    global _NC_CACHE, LAST_RESULT
    x = np.asarray(x, np.float32)
    Wq = np.asarray(Wq, np.float32)
    bq = np.asarray(bq, np.float32)
    Wk = np.asarray(Wk, np.float32)
    bk = np.asarray(bk, np.float32)
    v_embed = np.asarray(v_embed, np.float32)
    Wo = np.asarray(Wo, np.float32)
    bo = np.asarray(bo, np.float32)

    c = 2.0 * v_embed[:G].sum(axis=0)
    tri_m = (np.arange(TQ // NTQ)[None, :] >= np.arange(P)[:, None])

    in_maps = []
    for core in range(NCORES):
        b, g = divmod(core, G)
        gsl = slice(g * GF, (g + 1) * GF)
        # [D, X] arrays chunked as [P, D//P, X]: row d -> (d // 128 chunk
        # is INNER on partitions): layout "(n p) x -> p n x"
        xTc = np.ascontiguousarray(
            x[b].T.reshape(DK, P, T).transpose(1, 0, 2)
        ).astype(BF)
        xgc = np.ascontiguousarray(
            x[b][:, gsl].reshape(NTK, P, GF).transpose(1, 0, 2)
        ).astype(BF)
        wq_t = Wq[gsl, :].T  # [D, GF]
        wk_t = Wk[gsl, :].T
        wqk_np = np.stack([wq_t, wk_t], axis=1)  # [D, 2, GF]
        wqkc = np.ascontiguousarray(
            wqk_np.reshape(DK, P, 2, GF).transpose(1, 0, 2, 3)
        ).astype(BF)
        wo_t = Wo[:, gsl].T  # [GF, D]
        woc = np.ascontiguousarray(
            wo_t.reshape(GH, P, D).transpose(1, 0, 2)
        ).astype(BF)
        bqk = np.stack(
            [bq[gsl].reshape(GH, HD).T, bk[gsl].reshape(GH, HD).T], axis=1
        )  # [HD, 2, GH]
        in_maps.append(
            {
                "xT": xTc,
                "xg": xgc,
                "wqk": wqkc,
                "woT": woc,
                "cT": np.ascontiguousarray(c[gsl].reshape(GH, HD).T),
                "bqkT": np.ascontiguousarray(bqk),
                "ones": np.ones((P, 1), BF),
                "tri": tri_m.astype(BF),
            }
        )

    if _NC_CACHE is None:
        _NC_CACHE = build_program()
    res = run_bass_kernel_spmd(
        _NC_CACHE, in_maps, list(range(NCORES)), trace=TRACE
    )
    LAST_RESULT = res

    out = np.zeros((B, T, D), np.float32)
    for core in range(NCORES):
        b = core // G
        # out dram [P, DK, T] -> [D, T] -> [T, D]
        o = res.results[core]["out"].astype(np.float32)
        out[b] += o.transpose(1, 0, 2).reshape(D, T).T
    out += bo[None, None, :]
    return out


if __name__ == "__main__":
    nc = build_program()
    print("built ok")


# revision 10
# speedup vs baseline: 1.1284x; 1.0032x over previous
"""Trainium2 Bass kernel for nn_DMHA_3255585210402 (retrieval_knn DMHA).

Key algebraic fact: TOPK == NVK == 4, so jax.lax.top_k over the size-4 v_keys
axis selects *all* entries; the gather+sum over (DVH, TOPK) reduces to a
constant vector c = 2 * v_embed[0:4].sum(0), and compute_value_states
collapses to  v = x * c.

So the module is causal MHA (B=2, H=16, T=2048, HD=128, D=2048) with
elementwise-scaled V.  Sharding: 8 cores = 2 batches x 4 head-groups.

v2 design (vs v1):
  * all matmul operands bf16 (psum accumulation stays f32) - halves DMA
    and SBUF, and bf16 runs 1 cycle/row at ANY moving width (fp32r needs
    >=256), enabling fine-grained causal tiles.
  * diagonal 512-blocks computed at widths 512/384/256/128 instead of 4x512.
  * triangular mask applied by DVE tensor_mul with a [128,128] mask tile
    (gpsimd affine_select was on the exp->o-matmul critical path).
  * softmax denominators: off-diagonal chunks pair/quad-summed on DVE then
    one ones-matmul per quad; diagonal chunks get per-chunk ones-matmuls.
  * outproj psum->sbuf copies moved from Scalar to DVE so exp never queues
    behind them.
  * scores matmuls emitted with a 3-chunk skew ahead of o-matmuls to hide
    exp latency in the in-order PE queue.
  * DMA batching: Wq||Wk fused per-dk chunks (early), single-issue batched
    transfers for x (tci>=1), xg, Wo; output staged in SBUF and written as
    2 DMAs per query chunk (sync-engine DMA issue costs ~650ns each).
"""

import math

import numpy as np
import ml_dtypes

import concourse.bass as bass
import concourse.mybir as mybir
import concourse.tile as tile
from concourse import bacc
from concourse.bass_utils import run_bass_kernel_spmd

B, T, D = 2, 2048, 2048
H, HD = 16, 128
G = 4              # head-groups (cores per batch)
GH = H // G        # heads per core
GF = GH * HD       # projected features per core (512)
NCORES = 8
P = 128            # partitions
TQ = 512           # tq chunk width (psum bank / fp32 moving max)
F32 = mybir.dt.float32
BF16 = mybir.dt.bfloat16

DK = D // P        # 16 contraction chunks for projections
NTQ = T // TQ      # 4 query chunks
NTK = T // P       # 16 key chunks
SKEW = 3           # scores-ahead-of-o software pipeline depth

BF = ml_dtypes.bfloat16


def _body(tc, xT, xg, wqk, woT, cT, bqkT, ones, tri, out):
    nc = tc.nc
    rsqrt_hd = 1.0 / math.sqrt(HD)
    mult = mybir.AluOpType.mult

    with (
        tc.tile_pool(name="const", bufs=1) as const,
        tc.tile_pool(name="res1", bufs=1) as res1,
    ):
        # preload the gpsimd library that partition_broadcast needs so the
        # ~11us library DMA happens during phase A, not at first use
        from concourse import library_config
        with tc.high_priority():
            nc.gpsimd.load_library(library_config.attn)
        qT_sb = res1.tile([P, GH, T], BF16)   # q, transposed per head
        kT_sb = res1.tile([P, GH, T], BF16)
        # phase-B residents, DMA'd during phase A
        xg_sb = res1.tile([P, NTK, GF], BF16)   # x[:, gsl] chunked by tk
        wo_sb = res1.tile([P, GH, D], BF16)     # Wo[:, gsl].T chunked

        # --- phase A: q/k projections, transposed layout ---
        with (
            tc.tile_pool(name="wqk", bufs=1) as wqkp,
            tc.tile_pool(name="xt", bufs=2) as xtp,
            tc.tile_pool(name="psA", bufs=8, space="PSUM") as psA,
        ):
            wqk_sb = wqkp.tile([P, DK, 2, GF], BF16)
            xts = [xtp.tile([P, DK, TQ], BF16, name="xt") for _ in range(2)]
            # tci=0: per-dk chunked DMAs so the first matmul starts early
            for dk in range(DK):
                nc.sync.dma_start(out=wqk_sb[:, dk], in_=wqk[:, dk])
                nc.sync.dma_start(
                    out=xts[0][:, dk, :], in_=xT[:, dk, 0:TQ]
                )

            # small constants (single batched-issue DMAs)
            ones_sb = const.tile([P, 1], BF16)
            nc.sync.dma_start(out=ones_sb, in_=ones)
            tri_sb = const.tile([P, P], BF16)
            nc.sync.dma_start(out=tri_sb, in_=tri)
            bqk_sb = const.tile([HD, 2, GH], F32)
            nc.sync.dma_start(out=bqk_sb, in_=bqkT)
            cT_sb = const.tile([HD, GH], F32)
            nc.sync.dma_start(out=cT_sb, in_=cT)

            # batched prefetches for later phases (one issue each)
            nc.sync.dma_start(out=xts[1], in_=xT[:, :, TQ : 2 * TQ])
            nc.sync.dma_start(out=xg_sb, in_=xg)
            nc.sync.dma_start(out=wo_sb, in_=woT)

            for tci in range(NTQ):
                tsl = slice(tci * TQ, (tci + 1) * TQ)
                xt = xts[tci % 2]
                # q and k as separate 4-bank groups: banks free
                # incrementally, so phase B never waits on a full drain
                for w, dstT in ((0, qT_sb), (1, kT_sb)):
                    ps = [
                        psA.tile([P, TQ], F32, name="psA_t", tag="psA_t")
                        for _ in range(GH)
                    ]
                    for dk in range(DK):
                        for h in range(GH):
                            nc.tensor.matmul(
                                ps[h],
                                wqk_sb[:, dk, w, h * HD : (h + 1) * HD],
                                xt[:, dk, :],
                                start=(dk == 0),
                                stop=(dk == DK - 1),
                            )
                    if w == 1 and tci + 2 < NTQ:
                        # prefetch next x chunk (single batched issue);
                        # must come after BOTH halves have read xt
                        nsl = slice((tci + 2) * TQ, (tci + 3) * TQ)
                        nc.sync.dma_start(out=xt, in_=xT[:, :, nsl])
                    for h in range(GH):
                        nc.scalar.activation(
                            dstT[:, h, tsl],
                            ps[h],
                            mybir.ActivationFunctionType.Identity,
                            bias=bqk_sb[:, w, h : h + 1],
                        )

        # --- phases B+C interleaved over query chunks ---
        with (
            tc.tile_pool(name="res2", bufs=1) as res2,
            tc.tile_pool(name="wt", bufs=8) as wtp,
            tc.tile_pool(name="rb", bufs=2) as rbp,
            tc.tile_pool(name="pr", bufs=6) as prp,
            tc.tile_pool(name="small", bufs=4) as smp,
            tc.tile_pool(name="stg", bufs=2) as stgp,
            tc.tile_pool(name="psS", bufs=4, space="PSUM") as psS,
            tc.tile_pool(name="psO", bufs=2, space="PSUM") as psO,
            tc.tile_pool(name="psSum", bufs=2, space="PSUM") as psSum,
        ):
            oT_sb = res2.tile([P, GH, T], BF16)   # attention out, transposed

            pending = None
            stage_prev = None
            for j in range(NTQ):
                qsl = slice(j * TQ, (j + 1) * TQ)
                nkk = (j + 1) * (TQ // P)  # causal: tk chunks needed
                ndiag = TQ // P            # last 4 chunks are diagonal
                noff = nkk - ndiag
                for h in range(GH):
                    ps_o = psO.tile([P, TQ], F32, name="ps_o")
                    ps_sum = psSum.tile([1, TQ], F32, name="ps_sum")
                    wts = [None] * nkk
                    subs = [None] * nkk
                    emitted_o = 0

                    def emit_scores(i):
                        """scores matmul + exp (+ mask on diagonal)."""
                        g = i - noff
                        if g >= 0:
                            sub = slice(g * P, TQ)
                            w = TQ - g * P
                        else:
                            sub = slice(0, TQ)
                            w = TQ
                        ps_s = psS.tile(
                            [P, TQ], F32, name="ps_s", tag="ps_s"
                        )
                        nc.tensor.matmul(
                            ps_s[:, sub],
                            kT_sb[:, h, i * P : (i + 1) * P],
                            qT_sb[:, h, j * TQ + (TQ - w) : (j + 1) * TQ],
                            start=True,
                            stop=True,
                        )
                        wt = wtp.tile([P, TQ], BF16, name="wt")
                        nc.scalar.activation(
                            wt[:, sub], ps_s[:, sub],
                            mybir.ActivationFunctionType.Exp,
                            scale=rsqrt_hd,
                        )
                        if g >= 0:  # triangular mask on leading 128 cols
                            lead = slice(g * P, (g + 1) * P)
                            nc.vector.tensor_mul(
                                wt[:, lead], wt[:, lead], tri_sb
                            )
                        wts[i] = wt
                        subs[i] = sub

                    def emit_o(i):
                        """accumulate o-matmul + colsum contributions."""
                        sub = subs[i]
                        nc.tensor.matmul(
                            ps_o[:, sub],
                            xg_sb[:, i, h * HD : (h + 1) * HD],
                            wts[i][:, sub],
                            start=(i == 0),
                            stop=(i == nkk - 1),
                        )
                        g = i - noff
                        if g < 0:
                            # off-diagonal: quad-group for the colsum
                            if i % 4 == 3:
                                t0 = prp.tile([P, TQ], BF16, name="pr")
                                t1 = prp.tile([P, TQ], BF16, name="pr")
                                q0 = prp.tile([P, TQ], BF16, name="pr")
                                nc.vector.tensor_add(
                                    t0, wts[i - 3], wts[i - 2]
                                )
                                nc.vector.tensor_add(
                                    t1, wts[i - 1], wts[i]
                                )
                                nc.vector.tensor_add(q0, t0, t1)
                                nc.tensor.matmul(
                                    ps_sum, ones_sb, q0,
                                    start=(i == 3), stop=False,
                                )
                        else:
                            # diagonal: per-chunk ones-matmul at its width
                            nc.tensor.matmul(
                                ps_sum[:, sub], ones_sb, wts[i][:, sub],
                                start=(j == 0 and g == 0),
                                stop=(g == ndiag - 1),
                            )

                    for i in range(nkk):
                        emit_scores(i)
                        if i >= SKEW:
                            emit_o(emitted_o)
                            emitted_o += 1
                    while emitted_o < nkk:
                        emit_o(emitted_o)
                        emitted_o += 1

                    # normalization deferred one head so neither gpsimd nor
                    # the PE ever waits on the recip chain
                    if pending is not None:
                        _emit_normalize(nc, smp, rbp, oT_sb, cT_sb, mult,
                                        *pending)
                    pending = (h, j, ps_o, ps_sum)
                    # C(j-1) interleaved into B(j)'s head loop: spreads the
                    # PE-heavy/scalar-free outproj against scalar-heavy B
                    # (exp) and spreads the DVE casts.  h=0 is skipped so
                    # normalize(h3, j-1) lands first.
                    if j > 0 and h >= 1:
                        lo, hi = [(0, 6), (6, 11), (11, 16)][h - 1]
                        if h == 1:
                            stage = stgp.tile(
                                [P, DK, TQ], BF16, name="stage"
                            )
                        _emit_outproj(nc, psS, stage, wo_sb, oT_sb, out,
                                      j - 1, lo, hi)
            _emit_normalize(nc, smp, rbp, oT_sb, cT_sb, mult, *pending)
            stage = stgp.tile([P, DK, TQ], BF16, name="stage")
            _emit_outproj(nc, psS, stage, wo_sb, oT_sb, out, NTQ - 1, 0, DK)


def _emit_normalize(nc, smp, rbp, oT_sb, cT_sb, mult, h, j, ps_o, ps_sum):
    """1/colsum on one partition, gpsimd partition broadcast, then
    (ps_o * c[p]) * recip in one DVE pass."""
    qsl = slice(j * TQ, (j + 1) * TQ)
    recip = smp.tile([1, TQ], F32, name="recip")
    nc.vector.reciprocal_approx_fast(out=recip, in_=ps_sum)
    rb = rbp.tile([P, TQ], F32, name="rb")
    nc.gpsimd.partition_broadcast(rb, recip)
    nc.vector.scalar_tensor_tensor(
        out=oT_sb[:, h, qsl],
        in0=ps_o,
        scalar=cT_sb[:, h : h + 1],
        in1=rb,
        op0=mult,
        op1=mult,
    )


def _emit_outproj(nc, psS, stage, wo_sb, oT_sb, out, j, lo, hi):
    qsl = slice(j * TQ, (j + 1) * TQ)
    for dk in range(lo, hi):
        ps = psS.tile([P, TQ], F32, name="psC_t", tag="ps_s")
        for m in range(GH):
            nc.tensor.matmul(
                ps,
                wo_sb[:, m, dk * P : (dk + 1) * P],
                oT_sb[:, m, qsl],
                start=(m == 0),
                stop=(m == GH - 1),
            )
        nc.vector.tensor_copy(stage[:, dk, :], ps)
        # flush per 4-dk group so the final drain is at most ~1.6us
        if dk % 4 == 3:
            nc.sync.dma_start(
                out=out[:, dk - 3 : dk + 1, qsl],
                in_=stage[:, dk - 3 : dk + 1, :],
            )


def build_program():
    nc = bacc.Bacc(
        "TRN2", target_bir_lowering=False, debug=False, num_devices=NCORES
    )
    f = F32
    # xT: [128, DK, T] bf16 (feature-chunked, feature-on-partition)
    xT = nc.dram_tensor("xT", [P, DK, T], BF16, kind="ExternalInput").ap()
    # xg: [128, NTK, GF] bf16 (time-chunked slice of x for V)
    xg = nc.dram_tensor("xg", [P, NTK, GF], BF16, kind="ExternalInput").ap()
    # wqk: [128, DK, 2, GF] bf16 (fused Wq/Wk, transposed, chunked)
    wqk = nc.dram_tensor(
        "wqk", [P, DK, 2, GF], BF16, kind="ExternalInput"
    ).ap()
    # woT: [128, GH, D] bf16
    woT = nc.dram_tensor("woT", [P, GH, D], BF16, kind="ExternalInput").ap()
    cT = nc.dram_tensor("cT", [HD, GH], f, kind="ExternalInput").ap()
    bqkT = nc.dram_tensor("bqkT", [HD, 2, GH], f, kind="ExternalInput").ap()
    ones = nc.dram_tensor("ones", [P, 1], BF16, kind="ExternalInput").ap()
    tri = nc.dram_tensor("tri", [P, P], BF16, kind="ExternalInput").ap()
    # out: [128, DK, T] bf16 (row-chunked [D, T])
    out = nc.dram_tensor("out", [P, DK, T], BF16, kind="ExternalOutput").ap()

    with tile.TileContext(nc) as tc:
        _body(tc, xT, xg, wqk, woT, cT, bqkT, ones, tri, out)
    nc.compile()
    return nc


_NC_CACHE = None
LAST_RESULT = None
TRACE = False


def kernel(x, Wq, bq, Wk, bk, Wvq, bvq, v_keys, v_embed, Wo, bo):
    global _NC_CACHE, LAST_RESULT
    x = np.asarray(x, np.float32)
    Wq = np.asarray(Wq, np.float32)
    bq = np.asarray(bq, np.float32)
    Wk = np.asarray(Wk, np.float32)
    bk = np.asarray(bk, np.float32)
    v_embed = np.asarray(v_embed, np.float32)
    Wo = np.asarray(Wo, np.float32)
    bo = np.asarray(bo, np.float32)

    c = 2.0 * v_embed[:G].sum(axis=0)
    tri_m = (np.arange(TQ // NTQ)[None, :] >= np.arange(P)[:, None])

    in_maps = []
    for core in range(NCORES):
        b, g = divmod(core, G)
        gsl = slice(g * GF, (g + 1) * GF)
        # [D, X] arrays chunked as [P, D//P, X]: row d -> (d // 128 chunk
        # is INNER on partitions): layout "(n p) x -> p n x"
        xTc = np.ascontiguousarray(
            x[b].T.reshape(DK, P, T).transpose(1, 0, 2)
        ).astype(BF)
        xgc = np.ascontiguousarray(
            x[b][:, gsl].reshape(NTK, P, GF).transpose(1, 0, 2)
        ).astype(BF)
        wq_t = Wq[gsl, :].T  # [D, GF]
        wk_t = Wk[gsl, :].T
        wqk_np = np.stack([wq_t, wk_t], axis=1)  # [D, 2, GF]
        wqkc = np.ascontiguousarray(
            wqk_np.reshape(DK, P, 2, GF).transpose(1, 0, 2, 3)
        ).astype(BF)
        wo_t = Wo[:, gsl].T  # [GF, D]
        woc = np.ascontiguousarray(
            wo_t.reshape(GH, P, D).transpose(1, 0, 2)
        ).astype(BF)
        bqk = np.stack(
            [bq[gsl].reshape(GH, HD).T, bk[gsl].reshape(GH, HD).T], axis=1
        )  # [HD, 2, GH]
        in_maps.append(
            {
                "xT": xTc,
                "xg": xgc,
                "wqk": wqkc,
                "woT": woc,
                "cT": np.ascontiguousarray(c[gsl].reshape(GH, HD).T),
                "bqkT": np.ascontiguousarray(bqk),
                "ones": np.ones((P, 1), BF),
                "tri": tri_m.astype(BF),
            }
        )

    if _NC_CACHE is None:
        _NC_CACHE = build_program()
    res = run_bass_kernel_spmd(
        _NC_CACHE, in_maps, list(range(NCORES)), trace=TRACE
    )
    LAST_RESULT = res

    out = np.zeros((B, T, D), np.float32)
    for core in range(NCORES):
        b = core // G
        # out dram [P, DK, T] -> [D, T] -> [T, D]
        o = res.results[core]["out"].astype(np.float32)
        out[b] += o.transpose(1, 0, 2).reshape(D, T).T
    out += bo[None, None, :]
    return out


if __name__ == "__main__":
    nc = build_program()
    print("built ok")


# revision 15
# speedup vs baseline: 1.1652x; 1.0326x over previous
"""Trainium2 Bass kernel for nn_DMHA_3255585210402 (retrieval_knn DMHA).

Key algebraic fact: TOPK == NVK == 4, so jax.lax.top_k over the size-4 v_keys
axis selects *all* entries; the gather+sum over (DVH, TOPK) reduces to a
constant vector c = 2 * v_embed[0:4].sum(0), and compute_value_states
collapses to  v = x * c.

So the module is causal MHA (B=2, H=16, T=2048, HD=128, D=2048) with
elementwise-scaled V.  Sharding: 8 cores = 2 batches x 4 head-groups.

v2 design (vs v1):
  * all matmul operands bf16 (psum accumulation stays f32) - halves DMA
    and SBUF, and bf16 runs 1 cycle/row at ANY moving width (fp32r needs
    >=256), enabling fine-grained causal tiles.
  * diagonal 512-blocks computed at widths 512/384/256/128 instead of 4x512.
  * triangular mask applied by DVE tensor_mul with a [128,128] mask tile
    (gpsimd affine_select was on the exp->o-matmul critical path).
  * softmax denominators: off-diagonal chunks pair/quad-summed on DVE then
    one ones-matmul per quad; diagonal chunks get per-chunk ones-matmuls.
  * outproj psum->sbuf copies moved from Scalar to DVE so exp never queues
    behind them.
  * scores matmuls emitted with a 3-chunk skew ahead of o-matmuls to hide
    exp latency in the in-order PE queue.
  * DMA batching: Wq||Wk fused per-dk chunks (early), single-issue batched
    transfers for x (tci>=1), xg, Wo; output staged in SBUF and written as
    2 DMAs per query chunk (sync-engine DMA issue costs ~650ns each).
"""

import math

import numpy as np
import ml_dtypes

import concourse.bass as bass
import concourse.mybir as mybir
import concourse.tile as tile
from concourse import bacc
from concourse.bass_utils import run_bass_kernel_spmd

B, T, D = 2, 2048, 2048
H, HD = 16, 128
G = 4              # head-groups (cores per batch)
GH = H // G        # heads per core
GF = GH * HD       # projected features per core (512)
NCORES = 8
P = 128            # partitions
TQ = 512           # tq chunk width (psum bank / fp32 moving max)
F32 = mybir.dt.float32
BF16 = mybir.dt.bfloat16

DK = D // P        # 16 contraction chunks for projections
NTQ = T // TQ      # 4 query chunks
NTK = T // P       # 16 key chunks
SKEW = 3           # scores-ahead-of-o software pipeline depth

BF = ml_dtypes.bfloat16


def _body(tc, xT, xg, wqk, woT, cT, bqkT, ones, tri, out):
    nc = tc.nc
    rsqrt_hd = 1.0 / math.sqrt(HD)
    mult = mybir.AluOpType.mult

    with (
        tc.tile_pool(name="const", bufs=1) as const,
        tc.tile_pool(name="res1", bufs=1) as res1,
    ):
        # preload the gpsimd library that partition_broadcast needs so the
        # ~11us library DMA happens during phase A, not at first use
        from concourse import library_config
        with tc.high_priority():
            nc.gpsimd.load_library(library_config.attn)
        qT_sb = res1.tile([P, GH, T], BF16)   # q, transposed per head
        kT_sb = res1.tile([P, GH, T], BF16)
        # phase-B residents, DMA'd during phase A
        xg_sb = res1.tile([P, NTK, GF], BF16)   # x[:, gsl] chunked by tk
        wo_sb = res1.tile([P, GH, D], BF16)     # Wo[:, gsl].T chunked

        # --- phase A: q/k projections, transposed layout ---
        with (
            tc.tile_pool(name="wqk", bufs=1) as wqkp,
            tc.tile_pool(name="xt", bufs=2) as xtp,
            tc.tile_pool(name="psA", bufs=8, space="PSUM") as psA,
        ):
            wqk_sb = wqkp.tile([P, DK, 2, GF], BF16)
            xts = [xtp.tile([P, DK, TQ], BF16, name="xt") for _ in range(2)]
            ones_sb = const.tile([P, 1], BF16)
            tri_sb = const.tile([P, P], BF16)
            bqk_sb = const.tile([HD, 2, GH], F32)
            cT_sb = const.tile([HD, GH], F32)

            for tci in range(NTQ):
                tsl = slice(tci * TQ, (tci + 1) * TQ)
                xt = xts[tci % 2]
                # q and k as separate 4-bank groups: banks free
                # incrementally, so phase B never waits on a full drain
                for w, dstT in ((0, qT_sb), (1, kT_sb)):
                    ps = [
                        psA.tile([P, TQ], F32, name="psA_t", tag="psA_t")
                        for _ in range(GH)
                    ]
                    for dk in range(DK):
                        if tci == 0 and w == 0:
                            # JIT per-dk DMA issue: each dk's matmuls wait
                            # only on the DMAs issued so far, so the PE
                            # starts after ~2 transfers instead of ~6
                            nc.sync.dma_start(
                                out=wqk_sb[:, dk], in_=wqk[:, dk]
                            )
                            nc.sync.dma_start(
                                out=xts[0][:, dk, :], in_=xT[:, dk, 0:TQ]
                            )
                            if dk == 0:
                                nc.sync.dma_start(out=bqk_sb, in_=bqkT)
                            elif dk == 1:
                                nc.sync.dma_start(out=ones_sb, in_=ones)
                                nc.sync.dma_start(out=tri_sb, in_=tri)
                            elif dk == 2:
                                nc.sync.dma_start(out=cT_sb, in_=cT)
                        for h in range(GH):
                            nc.tensor.matmul(
                                ps[h],
                                wqk_sb[:, dk, w, h * HD : (h + 1) * HD],
                                xt[:, dk, :],
                                start=(dk == 0),
                                stop=(dk == DK - 1),
                            )
                    if tci == 0 and w == 0:
                        # batched prefetches for later phases, issued while
                        # the k-half runs off already-resident data
                        nc.sync.dma_start(
                            out=xts[1], in_=xT[:, :, TQ : 2 * TQ]
                        )
                        nc.sync.dma_start(out=xg_sb, in_=xg)
                        nc.sync.dma_start(out=wo_sb, in_=woT)
                    if w == 1 and tci + 2 < NTQ:
                        # prefetch next x chunk (single batched issue);
                        # must come after BOTH halves have read xt
                        nsl = slice((tci + 2) * TQ, (tci + 3) * TQ)
                        nc.sync.dma_start(out=xt, in_=xT[:, :, nsl])
                    for h in range(GH):
                        nc.scalar.activation(
                            dstT[:, h, tsl],
                            ps[h],
                            mybir.ActivationFunctionType.Identity,
                            bias=bqk_sb[:, w, h : h + 1],
                        )

        # --- phases B+C interleaved over query chunks ---
        with (
            tc.tile_pool(name="res2", bufs=1) as res2,
            tc.tile_pool(name="wt", bufs=10) as wtp,
            tc.tile_pool(name="rb", bufs=2) as rbp,
            tc.tile_pool(name="pr", bufs=6) as prp,
            tc.tile_pool(name="small", bufs=4) as smp,
            tc.tile_pool(name="stg", bufs=2) as stgp,
            tc.tile_pool(name="psS", bufs=4, space="PSUM") as psS,
            tc.tile_pool(name="psO", bufs=2, space="PSUM") as psO,
            tc.tile_pool(name="psSum", bufs=2, space="PSUM") as psSum,
        ):
            oT_sb = res2.tile([P, GH, T], BF16)   # attention out, transposed

            # Flat software pipeline over every (j, h, i) score unit with a
            # global skew: the exp-hiding lookahead never resets at head or
            # query-chunk boundaries, so the PE sees no dependency stalls
            # there.  Outproj chains for j-1 are injected mid-head.
            sunits = []
            for j in range(NTQ):
                for h in range(GH):
                    for i in range((j + 1) * (TQ // P)):
                        sunits.append((j, h, i))

            st = {}       # (j, h) -> [ps_o, ps_sum, wts]
            pending = None
            stage = None

            def emit_scores(j, h, i):
                nkk = (j + 1) * (TQ // P)
                g = i - (nkk - TQ // P)
                sub = slice(g * P, TQ) if g >= 0 else slice(0, TQ)
                ps_s = psS.tile([P, TQ], F32, name="ps_s", tag="ps_s")
                nc.tensor.matmul(
                    ps_s[:, sub],
                    kT_sb[:, h, i * P : (i + 1) * P],
                    qT_sb[:, h, j * TQ + sub.start : (j + 1) * TQ],
                    start=True,
                    stop=True,
                )
                wt = wtp.tile([P, TQ], BF16, name="wt")
                nc.scalar.activation(
                    wt[:, sub], ps_s[:, sub],
                    mybir.ActivationFunctionType.Exp,
                    scale=rsqrt_hd,
                )
                if g >= 0:  # triangular mask on leading 128 cols
                    lead = slice(g * P, (g + 1) * P)
                    nc.vector.tensor_mul(wt[:, lead], wt[:, lead], tri_sb)
                if i == 0:
                    st[(j, h)] = [None, None, {}]
                st[(j, h)][2][i] = (wt, sub)

            def emit_o(j, h, i):
                nonlocal pending, stage
                nkk = (j + 1) * (TQ // P)
                ndiag = TQ // P
                noff = nkk - ndiag
                ent = st[(j, h)]
                if i == 0:
                    ent[0] = psO.tile([P, TQ], F32, name="ps_o")
                    ent[1] = psSum.tile([1, TQ], F32, name="ps_sum")
                ps_o, ps_sum, wts = ent
                wt, sub = wts[i]
                nc.tensor.matmul(
                    ps_o[:, sub],
                    xg_sb[:, i, h * HD : (h + 1) * HD],
                    wt[:, sub],
                    start=(i == 0),
                    stop=(i == nkk - 1),
                )
                g = i - noff
                if g < 0:
                    if i % 4 == 3:  # off-diagonal quad colsum
                        t0 = prp.tile([P, TQ], BF16, name="pr")
                        t1 = prp.tile([P, TQ], BF16, name="pr")
                        q0 = prp.tile([P, TQ], BF16, name="pr")
                        nc.vector.tensor_add(t0, wts[i - 3][0], wts[i - 2][0])
                        nc.vector.tensor_add(t1, wts[i - 1][0], wts[i][0])
                        nc.vector.tensor_add(q0, t0, t1)
                        nc.tensor.matmul(
                            ps_sum, ones_sb, q0,
                            start=(i == 3), stop=False,
                        )
                else:
                    # diagonal: per-chunk ones-matmul at its width
                    nc.tensor.matmul(
                        ps_sum[:, sub], ones_sb, wt[:, sub],
                        start=(j == 0 and g == 0),
                        stop=(g == ndiag - 1),
                    )
                if i == nkk - 1:
                    # head complete: flush the deferred normalize, defer ours
                    if pending is not None:
                        _emit_normalize(nc, smp, rbp, oT_sb, cT_sb, mult,
                                        *pending)
                    pending = (h, j, ps_o, ps_sum)
                    del st[(j, h)]
                if j > 0 and h >= 1 and i == 1:
                    # outproj chains for j-1, spread across heads 1..3
                    lo, hi = [(0, 6), (6, 11), (11, 16)][h - 1]
                    if h == 1:
                        stage = stgp.tile([P, DK, TQ], BF16, name="stage")
                    _emit_outproj(nc, psS, stage, wo_sb, oT_sb, out,
                                  j - 1, lo, hi)

            for u, (j, h, i) in enumerate(sunits):
                emit_scores(j, h, i)
                if u >= SKEW:
                    emit_o(*sunits[u - SKEW])
            for u in range(len(sunits) - SKEW, len(sunits)):
                emit_o(*sunits[u])
            _emit_normalize(nc, smp, rbp, oT_sb, cT_sb, mult, *pending)
            stage = stgp.tile([P, DK, TQ], BF16, name="stage")
            _emit_outproj(nc, psS, stage, wo_sb, oT_sb, out, NTQ - 1, 0, DK)


def _emit_normalize(nc, smp, rbp, oT_sb, cT_sb, mult, h, j, ps_o, ps_sum):
    """1/colsum on one partition, gpsimd partition broadcast, then
    (ps_o * c[p]) * recip in one DVE pass."""
    qsl = slice(j * TQ, (j + 1) * TQ)
    recip = smp.tile([1, TQ], F32, name="recip")
    nc.vector.reciprocal_approx_fast(out=recip, in_=ps_sum)
    rb = rbp.tile([P, TQ], F32, name="rb")
    nc.gpsimd.partition_broadcast(rb, recip)
    nc.vector.scalar_tensor_tensor(
        out=oT_sb[:, h, qsl],
        in0=ps_o,
        scalar=cT_sb[:, h : h + 1],
        in1=rb,
        op0=mult,
        op1=mult,
    )


def _emit_outproj(nc, psS, stage, wo_sb, oT_sb, out, j, lo, hi):
    qsl = slice(j * TQ, (j + 1) * TQ)
    for dk in range(lo, hi):
        ps = psS.tile([P, TQ], F32, name="psC_t", tag="ps_s")
        for m in range(GH):
            nc.tensor.matmul(
                ps,
                wo_sb[:, m, dk * P : (dk + 1) * P],
                oT_sb[:, m, qsl],
                start=(m == 0),
                stop=(m == GH - 1),
            )
        nc.vector.tensor_copy(stage[:, dk, :], ps)
        # flush in groups, smaller at the end so the final drain is short
        if dk in (3, 7, 11, 13, 15):
            flo = {3: 0, 7: 4, 11: 8, 13: 12, 15: 14}[dk]
            nc.sync.dma_start(
                out=out[:, flo : dk + 1, qsl],
                in_=stage[:, flo : dk + 1, :],
            )


def build_program():
    nc = bacc.Bacc(
        "TRN2", target_bir_lowering=False, debug=False, num_devices=NCORES
    )
    f = F32
    # xT: [128, DK, T] bf16 (feature-chunked, feature-on-partition)
    xT = nc.dram_tensor("xT", [P, DK, T], BF16, kind="ExternalInput").ap()
    # xg: [128, NTK, GF] bf16 (time-chunked slice of x for V)
    xg = nc.dram_tensor("xg", [P, NTK, GF], BF16, kind="ExternalInput").ap()
    # wqk: [128, DK, 2, GF] bf16 (fused Wq/Wk, transposed, chunked)
    wqk = nc.dram_tensor(
        "wqk", [P, DK, 2, GF], BF16, kind="ExternalInput"
    ).ap()
    # woT: [128, GH, D] bf16
    woT = nc.dram_tensor("woT", [P, GH, D], BF16, kind="ExternalInput").ap()
    cT = nc.dram_tensor("cT", [HD, GH], f, kind="ExternalInput").ap()
    bqkT = nc.dram_tensor("bqkT", [HD, 2, GH], f, kind="ExternalInput").ap()
    ones = nc.dram_tensor("ones", [P, 1], BF16, kind="ExternalInput").ap()
    tri = nc.dram_tensor("tri", [P, P], BF16, kind="ExternalInput").ap()
    # out: [128, DK, T] bf16 (row-chunked [D, T])
    out = nc.dram_tensor("out", [P, DK, T], BF16, kind="ExternalOutput").ap()

    with tile.TileContext(nc) as tc:
        _body(tc, xT, xg, wqk, woT, cT, bqkT, ones, tri, out)
    nc.compile()
    return nc


_NC_CACHE = None
LAST_RESULT = None
TRACE = False


def kernel(x, Wq, bq, Wk, bk, Wvq, bvq, v_keys, v_embed, Wo, bo):
    global _NC_CACHE, LAST_RESULT
    x = np.asarray(x, np.float32)
    Wq = np.asarray(Wq, np.float32)
    bq = np.asarray(bq, np.float32)
    Wk = np.asarray(Wk, np.float32)
    bk = np.asarray(bk, np.float32)
    v_embed = np.asarray(v_embed, np.float32)
    Wo = np.asarray(Wo, np.float32)
    bo = np.asarray(bo, np.float32)

    c = 2.0 * v_embed[:G].sum(axis=0)
    tri_m = (np.arange(TQ // NTQ)[None, :] >= np.arange(P)[:, None])

    in_maps = []
    for core in range(NCORES):
        b, g = divmod(core, G)
        gsl = slice(g * GF, (g + 1) * GF)
        # [D, X] arrays chunked as [P, D//P, X]: row d -> (d // 128 chunk
        # is INNER on partitions): layout "(n p) x -> p n x"
        xTc = np.ascontiguousarray(
            x[b].T.reshape(DK, P, T).transpose(1, 0, 2)
        ).astype(BF)
        xgc = np.ascontiguousarray(
            x[b][:, gsl].reshape(NTK, P, GF).transpose(1, 0, 2)
        ).astype(BF)
        wq_t = Wq[gsl, :].T  # [D, GF]
        wk_t = Wk[gsl, :].T
        wqk_np = np.stack([wq_t, wk_t], axis=1)  # [D, 2, GF]
        wqkc = np.ascontiguousarray(
            wqk_np.reshape(DK, P, 2, GF).transpose(1, 0, 2, 3)
        ).astype(BF)
        wo_t = Wo[:, gsl].T  # [GF, D]
        woc = np.ascontiguousarray(
            wo_t.reshape(GH, P, D).transpose(1, 0, 2)
        ).astype(BF)
        bqk = np.stack(
            [bq[gsl].reshape(GH, HD).T, bk[gsl].reshape(GH, HD).T], axis=1
        )  # [HD, 2, GH]
        in_maps.append(
            {
                "xT": xTc,
                "xg": xgc,
                "wqk": wqkc,
                "woT": woc,
                "cT": np.ascontiguousarray(c[gsl].reshape(GH, HD).T),
                "bqkT": np.ascontiguousarray(bqk),
                "ones": np.ones((P, 1), BF),
                "tri": tri_m.astype(BF),
            }
        )

    if _NC_CACHE is None:
        _NC_CACHE = build_program()
    res = run_bass_kernel_spmd(
        _NC_CACHE, in_maps, list(range(NCORES)), trace=TRACE
    )
    LAST_RESULT = res

    out = np.zeros((B, T, D), np.float32)
    for core in range(NCORES):
        b = core // G
        # out dram [P, DK, T] -> [D, T] -> [T, D]
        o = res.results[core]["out"].astype(np.float32)
        out[b] += o.transpose(1, 0, 2).reshape(D, T).T
    out += bo[None, None, :]
    return out


if __name__ == "__main__":
    nc = build_program()
    print("built ok")
